# revision 13
# baseline (speedup 1.0000x reference)
"""Trainium2 Bass kernel for nn_Attention_23699629539900 — v2 (fp16, overlap).

Data-parallel over batch: 8 cores, one batch element each, no collectives.
All matmuls fp16 x fp16 -> f32 PSUM (same 1.0 cyc/row rate as fp32r, half
the DMA/SBUF; fp16's 10-bit mantissa keeps rel err ~1e-2 under the 2e-2
gate where bf16 was ~3e-2). PE is the bottleneck (~190us of matmul
streaming); the schedule keeps the PE queue fed at all times:

- dots PSUM double-buffered so QK(kc+1) never waits exp(kc); exp is one
  fused ACT call per kc step (2 heads x 512 queries)
- AV lhsT slots are [v(64)|ones(64)] blocks; out rows 0:64 = out.T,
  rows 64:128 = softmax denominator (ones trick); the accumulator is
  evacuated to SBUF in one copy so the PSUM bank frees early
- attention groups stream through a cross-group software pipeline (a
  group's last AVs + divide run under the next group's QKs); ph1 qk
  groups, vectorizer layers and cv-final chunks are interleaved into the
  stream as PE filler, each emitted just ahead of its consumer
- sumsq scratch stays f32 (fp16 subnormal-flush would zero tiny squares)
"""
import os
import numpy as np
from contextlib import ExitStack

import concourse.bass as bass
import concourse.tile as tile
from concourse import bacc, mybir
from concourse.bass_utils import run_bass_kernel_spmd

F32 = mybir.dt.float32
BF16 = mybir.dt.float16
AF = mybir.ActivationFunctionType
ALU = mybir.AluOpType
NPBF16 = mybir.dt.np(BF16)

B, N, D = 8, 1024, 512
H, DH = 8, 64
CH, CD = 6, 64
ID = H * DH
SCALE = DH ** -0.5
COLS = {"nlb": 0, "bout": 4, "negsub": 8,
        "ckb0": 12, "ckb1": 16, "ckb2": 20, "cvb0": 24, "cvb1": 28}


def _declare_inputs(nc):
    t = {}
    def inp(name, shape, dt=BF16):
        t[name] = nc.dram_tensor(name, list(shape), dt, kind="ExternalInput").ap()
    inp("xT", (D, N))
    inp("ckT", (CH * CD, N))
    inp("cvT", (CH * CD, N))
    inp("wqkvT", (D, 3 * ID))
    inp("ckw0T", (CH * CD, ID)); inp("ckw1T", (ID, ID)); inp("ckw2T", (ID, ID))
    inp("cvw0T", (CH * CD, ID)); inp("cvw1T", (ID, ID)); inp("cvw2T", (ID, ID))
    inp("nlwT", (ID, ID)); inp("woutT", (D, ID))
    inp("rows", (1, 512))               # cvb2 free-direction bias row
    inp("smallcols", (128, 32), F32)    # bias/scale columns
    inp("ones_blk", (128, 512))         # bf16 ones
    t["outT"] = nc.dram_tensor("outT", [D, N], F32, kind="ExternalOutput").ap()
    return t


def build_nc():
    nc = bacc.Bacc("TRN2", target_bir_lowering=False, debug=False, num_devices=8)
    t = _declare_inputs(nc)

    with tile.TileContext(nc) as tc, ExitStack() as ctx:
        const = ctx.enter_context(tc.tile_pool(name="const", bufs=1))
        io = ctx.enter_context(tc.tile_pool(name="io", bufs=1))
        vect = ctx.enter_context(tc.tile_pool(name="vect", bufs=1))
        wp = ctx.enter_context(tc.tile_pool(name="wp", bufs=3))
        ep = ctx.enter_context(tc.tile_pool(name="ep", bufs=6))
        rp = ctx.enter_context(tc.tile_pool(name="rp", bufs=1))
        avcp = ctx.enter_context(tc.tile_pool(name="avcp", bufs=2))
        pd = ctx.enter_context(tc.tile_pool(name="pd", bufs=2, space="PSUM"))
        pav = ctx.enter_context(tc.tile_pool(name="pav", bufs=1, space="PSUM"))
        pq = ctx.enter_context(tc.tile_pool(name="pq", bufs=2, space="PSUM"))

        # ---------------- constants (gpsimd queue; sync/scalar carry x/wqkv)
        rows = const.tile([1, 512], BF16)
        ones_blk = const.tile([128, 512], BF16)
        smallcols = const.tile([128, 32], F32)
        nc.gpsimd.dma_start(ones_blk[:], t["ones_blk"][:])
        bcol = lambda nm: smallcols[:, COLS[nm]:COLS[nm] + 4]
        stats = const.tile([128, 16], F32)   # ck 0:3 | cv 4:7 | mlp 8:12
        acc_scr = const.tile([128, 1024], F32, name="acc_scr")  # sq scratch (f32: fp16 subnormal-flush would zero tiny squares)

        # ---------------- long-lived tiles ----------------
        xT = io.tile([128, 4, N], BF16, name="xT")
        wqkv = io.tile([128, 4, 3 * ID], BF16, name="wqkv")
        q = io.tile([128, 4, N], BF16, name="q")
        k = io.tile([128, 4, N], BF16, name="k")
        # v / cvh tables: slot s = kc*8+h is a [128, 128] block [v(64)|ones(64)]
        v_st = io.tile([128, 64, 128], BF16, name="v_st", tag="vst")
        cvh_st = io.tile([128, 64, 128], BF16, name="cvh_st")
        outT_std = io.tile([128, 4, N], BF16, name="outT_std")
        outT_ctx = io.tile([128, 4, N], BF16, name="outT_ctx")
        ckh = io.tile([128, 4, N], BF16, name="ckh")
        # osb written only after v_st's last read (std g7 AVs)
        osb = io.tile([128, 4, N], F32, name="osb", tag="vst")

        cin_ck = vect.tile([128, 3, N], BF16, name="cin_ck", tag="cin_ck")
        cin_cv = vect.tile([128, 3, N], BF16, name="cin_cv", tag="cin_cv")
        y0 = vect.tile([128, 4, N], BF16, name="y0")
        y1 = vect.tile([128, 4, N], BF16, name="y1")
        mlp_in = vect.tile([128, 4, N], BF16, name="mlp_in", tag="cin_ck")
        mlpT = vect.tile([128, 4, N], BF16, name="mlpT", tag="cin_cv")
        comb = vect.tile([128, 4, N], BF16, name="comb")
        nlw = vect.tile([128, 4, ID], BF16, name="nlw")
        wout = vect.tile([128, 4, ID], BF16, name="wout")
        tmp_pr = vect.tile([128, 1024], BF16, name="tmp_pr")  # prelu scratch

        # ---------------- input DMAs (priority order) ----------------
        wq_r = t["wqkvT"].rearrange("(c p) f -> p c f", p=128)
        xT_r = t["xT"].rearrange("(c p) n -> p c n", p=128)
        # minimal DMA count on the critical path, split across two queues:
        # sync: x then v-cols; scalar: q-cols then k-cols
        nc.sync.dma_start(xT[:, 0:2, :], xT_r[:, 0:2, :])
        nc.scalar.dma_start(wqkv[:, :, 0:128], wq_r[:, :, 0:128])
        nc.sync.dma_start(xT[:, 2:4, :], xT_r[:, 2:4, :])
        nc.scalar.dma_start(wqkv[:, :, 128:ID], wq_r[:, :, 128:ID])
        nc.sync.dma_start(wqkv[:, :, 2 * ID:3 * ID], wq_r[:, :, 2 * ID:3 * ID])
        nc.scalar.dma_start(wqkv[:, :, ID:2 * ID], wq_r[:, :, ID:2 * ID])
        nc.scalar.dma_start(cin_ck[:], t["ckT"].rearrange("(c p) n -> p c n", p=128))
        nc.scalar.dma_start(cin_cv[:], t["cvT"].rearrange("(c p) n -> p c n", p=128))
        nc.gpsimd.dma_start(smallcols[:], t["smallcols"][:])
        nc.gpsimd.dma_start(rows[:], t["rows"][:])


        def loadw(name, nkk, eng=None):
            w = wp.tile([128, nkk, ID], BF16, name=name + "_t", tag="w")
            (eng or nc.gpsimd).dma_start(
                w[:], t[name].rearrange("(c p) f -> p c f", p=128))
            return w

        # ---------------- ph1: q/k feature-major, v token-major ----------------
        def qk_group(m):
            ps = pd.tile([128, 1024], F32, tag="pd", name=f"qk_ps{m}")
            for qt in range(2):
                s = slice(qt * 512, (qt + 1) * 512)
                for kk in range(4):
                    nc.tensor.matmul(ps[:, s], wqkv[:, kk, m * 128:(m + 1) * 128],
                                     xT[:, kk, s], start=(kk == 0), stop=(kk == 3))
            dst = q if m < 4 else k
            nc.vector.tensor_copy(dst[:, m % 4, :], ps[:])

        def v_group(t8, pool):
            for half, tt in enumerate((t8, t8 + 1)):
                if pool is pav:
                    big = pav.tile([128, 1024], F32, tag="pav",
                                   name=f"v_ps{tt}")
                    ps = big[:, 0:512]
                else:
                    ps = pq.tile([128, 512], F32, tag="pq", name=f"v_ps{tt}")[:]
                for kk in range(4):
                    nc.tensor.matmul(ps, xT[:, kk, tt * 128:(tt + 1) * 128],
                                     wqkv[:, kk, 2 * ID:3 * ID],
                                     start=(kk == 0), stop=(kk == 3))
                nc.vector.tensor_copy(
                    v_st[:, tt * 8:tt * 8 + 8, 0:64],
                    ps.rearrange("p (h d) -> p h d", h=H))

        # emission: q0/k0 + all v groups up front (std g0 needs them); the
        # remaining qk groups are interleaved into the stream as PE filler
        qk_group(0)
        qk_group(4)
        v_group(0, pav)
        # ones columns of both v tables (denominator trick), placed after
        # the first evictions so they don't head-of-line-block ph1
        for kc in range(8):
            nc.vector.tensor_copy(
                v_st[:, kc * 8:(kc + 1) * 8, 64:128],
                ones_blk[:].rearrange("p (h d) -> p h d", h=H))
        v_group(2, pq)
        for kc in range(8):
            nc.vector.tensor_copy(
                cvh_st[:, kc * 8:(kc + 1) * 8, 64:128],
                ones_blk[:].rearrange("p (h d) -> p h d", h=H))
        v_group(4, pav)
        v_group(6, pq)

        # ---------------- cin l2norm (over tokens) ----------------
        def cin_norm(cin, c0):
            # DVE square+reduce (ttr faults on HW): the DVE queue is empty
            # this early, keeping ACT free for the ph1 evictions
            for c in range(3):
                nc.vector.tensor_mul(acc_scr[:], cin[:, c, :], cin[:, c, :])
                nc.vector.reduce_sum(stats[:, c0 + c:c0 + c + 1], acc_scr[:],
                                     axis=mybir.AxisListType.X)

        def cin_stats(c0):
            nc.scalar.activation(stats[:, c0:c0 + 3], stats[:, c0:c0 + 3],
                                 AF.Sqrt, bias=0.0, scale=1.0)
            nc.vector.tensor_scalar_max(stats[:, c0:c0 + 3],
                                        stats[:, c0:c0 + 3], 1e-12)
            nc.vector.reciprocal(stats[:, c0:c0 + 3], stats[:, c0:c0 + 3])

        def cin_apply(cin, c0, c):
            nc.vector.tensor_scalar_mul(cin[:, c, :], cin[:, c, :],
                                        stats[:, c0 + c:c0 + c + 1])

        cin_norm(cin_ck, 0)
        cin_norm(cin_cv, 4)
        cin_stats(0)
        cin_stats(4)
        for c in range(3):
            cin_apply(cin_ck, 0, c)
        for c in range(3):
            cin_apply(cin_cv, 4, c)

        # ---------------- attention group (split for kc-level interleave) ----
        def attn_state(tag, p, qt, kT_get, v_tile, out_tile, av_pool, av_tag):
            av = av_pool.tile([128, 1024], F32, tag=av_tag,
                              name=f"{tag}av{p}_{qt}")
            return dict(tag=tag, p=p, qt=qt, kT_get=kT_get, v_tile=v_tile,
                        out_tile=out_tile, av=av,
                        qs=slice(qt * 512, qt * 512 + 512))

        def attn_qk(st, kc):
            tag, p, qs = st["tag"], st["p"], st["qs"]
            h0, h1 = 2 * p, 2 * p + 1
            d = pd.tile([128, 1024], F32, tag="pd",
                        name=f"{tag}d{p}_{st['qt']}_{kc}")
            nc.tensor.matmul(d[:, 0:512], st["kT_get"](h0, kc), q[0:64, p, qs],
                             start=True, stop=True)
            nc.tensor.matmul(d[:, 512:1024], st["kT_get"](h1, kc),
                             q[64:128, p, qs], start=True, stop=True)
            E = ep.tile([128, 1024], BF16, tag="E",
                        name=f"{tag}E{p}_{st['qt']}_{kc}")
            nc.scalar.activation(E[:], d[:], AF.Exp, bias=0.0, scale=SCALE)
            st["E" + str(kc)] = E

        def attn_av(st, kc):
            p = st["p"]
            E = st.pop("E" + str(kc))
            for i, h in enumerate((2 * p, 2 * p + 1)):
                lhsT = st["v_tile"][:, kc * 8 + h, :]  # [128, 128] = v|ones
                nc.tensor.matmul(st["av"][:, i * 512:(i + 1) * 512], lhsT,
                                 E[:, i * 512:(i + 1) * 512],
                                 start=(kc == 0), stop=(kc == 7),
                                 skip_group_check=True)

        def attn_fin(st):
            tag, p, qs, av = st["tag"], st["p"], st["qs"], st["av"]
            out_tile = st["out_tile"]
            # evacuate the PSUM accumulator in one copy so the bank frees
            # ~1.2us earlier; recip/muls run off the SBUF copy off-path
            cp = avcp.tile([128, 1024], F32, tag="avcp",
                           name=f"{tag}cp{p}_{st['qt']}")
            nc.vector.tensor_copy(cp[:], av[:])
            r = rp.tile([64, 1024], F32, tag="r", name=f"{tag}r{p}_{st['qt']}")
            nc.vector.reciprocal(r[:], cp[64:128, :])
            nc.vector.tensor_mul(out_tile[0:64, p, qs], cp[0:64, 0:512],
                                 r[:, 0:512])
            nc.vector.tensor_mul(out_tile[64:128, p, qs], cp[0:64, 512:1024],
                                 r[:, 512:1024])

        def attn_stream(states, fin_hooks=None, group_hooks=None,
                        step_hooks=None):
            """Cross-group two-deep software pipeline: group g's AV(kc) trails
            its QK(kc) by 2 steps; AV(6), AV(7) and the divide of group g run
            under group g+1's first QKs so the PE queue never drains at a
            group boundary."""
            prev = None
            for gi, make_st in enumerate(states):
                st = make_st()
                for kc in range(8):
                    attn_qk(st, kc)
                    if prev is not None and kc < 4:
                        attn_av(prev, 4 + kc)
                        if kc == 3:
                            attn_fin(prev)
                            if fin_hooks:
                                fin_hooks(prev)
                    if kc >= 4:
                        attn_av(st, kc - 4)
                    if step_hooks:
                        step_hooks(gi, kc)
                prev = st
                if group_hooks:
                    group_hooks(gi)
            for kc in range(4, 8):
                attn_av(prev, kc)
            attn_fin(prev)
            if fin_hooks:
                fin_hooks(prev)

        k_get = lambda h, kc: k[(h % 2) * 64:(h % 2) * 64 + 64, h // 2,
                                kc * 128:(kc + 1) * 128]
        ckh_get = lambda h, kc: ckh[(h % 2) * 64:(h % 2) * 64 + 64, h // 2,
                                    kc * 128:(kc + 1) * 128]

        # ---------------- vectorizer pieces ----------------
        def prelu_dve(out_ap, ps_ap, bias_ap, fd):
            # lrelu(ps + bias): tmp = (ps + b)*0.2; out = (ps + b) max tmp
            nc.vector.tensor_scalar(out=tmp_pr[:, 0:fd], in0=ps_ap,
                                    scalar1=bias_ap, scalar2=0.2,
                                    op0=ALU.add, op1=ALU.mult)
            nc.vector.scalar_tensor_tensor(
                out=out_ap, in0=ps_ap, scalar=bias_ap, in1=tmp_pr[:, 0:fd],
                op0=ALU.add, op1=ALU.max)

        def vect_layer_step(wtile, nkk, m, src, dst, bias_nm, nm, on_act):
            """one m-group of a feature-major vectorizer layer, emitted as
            two ping-ponging qt-halves so the prelu of one half overlaps the
            matmuls of the other"""
            for qt in range(2):
                s = slice(qt * 512, (qt + 1) * 512)
                ps = pq.tile([128, 512], F32, tag="pq", name=f"{nm}_ps{m}_{qt}")
                for kk in range(nkk):
                    nc.tensor.matmul(ps[:],
                                     wtile[:, kk, m * 128:(m + 1) * 128],
                                     src[:, kk, s], start=(kk == 0),
                                     stop=(kk == nkk - 1))
                if on_act:
                    nc.scalar.activation(dst[:, m, s], ps[:], AF.Prelu,
                                         bias=bcol(bias_nm)[:, m:m + 1],
                                         scale=1.0, alpha=0.2)
                else:
                    prelu_dve(dst[:, m, s], ps[:],
                              bcol(bias_nm)[:, m:m + 1], 512)

        def cv_final_step(t8):
            """token-major final cv layer -> cvh_st slots"""
            ps = pq.tile([128, 512], F32, tag="pq", name=f"cvf_ps{t8}")
            for kk in range(4):
                nc.tensor.matmul(ps[:], y1[:, kk, t8 * 128:(t8 + 1) * 128],
                                 wcv2[:, kk, :], start=(kk == 0), stop=False)
            nc.tensor.matmul(ps[:], ones_blk[0:1, 0:128], rows[0:1, :],
                             start=False, stop=True)
            nc.vector.tensor_scalar_mul(tmp_pr[:, 0:512], ps[:], 0.2)
            nc.vector.tensor_tensor(
                out=cvh_st[:, t8 * 8:t8 * 8 + 8, 0:64],
                in0=ps[:].rearrange("p (h d) -> p h d", h=H),
                in1=tmp_pr[:, 0:512].rearrange("p (h d) -> p h d", h=H),
                op=ALU.max)

        # weight loads early (gpsimd queue, after cin DMAs)
        wck0 = loadw("ckw0T", 3)
        wcv0 = loadw("cvw0T", 3)
        wck1 = loadw("ckw1T", 4)
        wcv1 = loadw("cvw1T", 4)
        wck2 = loadw("ckw2T", 4)
        wcv2 = loadw("cvw2T", 4)
        nc.gpsimd.dma_start(nlw[:], t["nlwT"].rearrange("(c p) f -> p c f", p=128))
        nc.gpsimd.dma_start(wout[:], t["woutT"].rearrange("(c p) f -> p c f", p=128))

        # ---------------- std attention + vect interleave ----------------
        # vect filler steps in dependency order, interleaved between groups
        filler = [("qk", 1), ("qk", 5), ("qk", 2), ("qk", 6),
                  ("qk", 3), ("qk", 7)]
        for m in range(4):
            filler.append(("ck0", m))
        for m in range(4):
            filler.append(("cv0", m))
        for m in range(4):
            filler.append(("ck1", m))
        filler.append(("cv1", 0))
        filler.append(("ck2", 0))
        for m in range(1, 4):
            filler.append(("cv1", m))
        filler.append(("cvf", 0))
        # ck2 m1-3 and cvf 1-7 are trickled into the ctx window as PE cover,
        # each just ahead of its consumer (ckh chunk p / cvh kc-slot)

        no_act_prelu = bool(os.environ.get("K2_NO_ACT_PRELU"))
        sim_gelu = (AF.Tanh if os.environ.get("K2_NO_ACT_PRELU")
                    else AF.Gelu)

        def do_filler(item):
            kind, m = item
            # alternate prelu engine to balance queues (CoreSim lacks Prelu)
            on_act = (m % 2 == 0) and not no_act_prelu
            if kind == "qk":
                qk_group(m)
            elif kind == "ck0":
                vect_layer_step(wck0, 3, m, cin_ck, y0, "ckb0", "ck0", on_act)
            elif kind == "cv0":
                vect_layer_step(wcv0, 3, m, cin_cv, y0cv, "cvb0", "cv0", on_act)
            elif kind == "ck1":
                vect_layer_step(wck1, 4, m, y0, y1ck, "ckb1", "ck1", on_act)
            elif kind == "cv1":
                vect_layer_step(wcv1, 4, m, y0cv, y1, "cvb1", "cv1", on_act)
            elif kind == "ck2":
                vect_layer_step(wck2, 4, m, y1ck, ckh, "ckb2", "ck2", on_act)
            elif kind == "cvf":
                cv_final_step(m)

        # extra tiles for the two parallel chains
        y0cv = vect.tile([128, 4, N], BF16, name="y0cv")
        y1ck = vect.tile([128, 4, N], BF16, name="y1ck")

        fi = 0
        def emit_filler(n):
            nonlocal fi
            for _ in range(n):
                if fi < len(filler):
                    do_filler(filler[fi])
                    fi += 1

        groups = [(p, qt) for p in range(4) for qt in range(2)]

        def mk_std(g):
            return lambda: attn_state("s", *groups[g], k_get, v_st, outT_std,
                                      pav, "pav")

        def mk_ctx(g):
            return lambda: attn_state("c", *groups[g], ckh_get, cvh_st,
                                      outT_ctx, pav, "pav")

        def fin_hooks(st):
            # after ctx pair (p, qt=1): chunk p of outT_ctx is complete ->
            # sumsq for the mlp l2norm
            if st["tag"] == "c" and st["qt"] == 1:
                p = st["p"]
                nc.vector.tensor_mul(acc_scr[:], outT_ctx[:, p, :],
                                     outT_ctx[:, p, :])
                nc.vector.reduce_sum(stats[:, 8 + p:9 + p], acc_scr[:],
                                     axis=mybir.AxisListType.X)

        # one pipelined stream: std g0-g5 (with vect filler between groups),
        # then ctx groups with std g6 injected mid-window as extra PE cover.
        # std g7 is saved for the mlp-stats window.
        stream = ([mk_std(g) for g in range(6)]
                  + [mk_ctx(g) for g in range(6)] + [mk_std(6)]
                  + [mk_ctx(6), mk_ctx(7)])
        w3_extra = {7: ("ck2", 1), 9: ("ck2", 2), 11: ("ck2", 3)}

        def group_hooks(gi):
            emit_filler(4)
            if gi in w3_extra:
                do_filler(w3_extra[gi])

        def step_hooks(gi, kc):
            if gi == 6 and kc < 7:
                do_filler(("cvf", kc + 1))

        attn_stream(stream, fin_hooks=fin_hooks, group_hooks=group_hooks,
                    step_hooks=step_hooks)

        # ---------------- mlp ----------------
        # stats chain + applies trickled through std g7's steps so the ACT
        # exp stream and the chain's latency overlap g7's matmuls
        def mlp_stats_hook(gi, kc):
            if kc == 3:
                nc.scalar.activation(stats[:, 8:12], stats[:, 8:12], AF.Sqrt,
                                     bias=0.0, scale=1.0)
                nc.vector.reciprocal(stats[:, 8:12], stats[:, 8:12])
            elif kc >= 4:
                c = kc - 4
                nc.vector.tensor_scalar_mul(mlp_in[:, c, :], outT_ctx[:, c, :],
                                            stats[:, 8 + c:9 + c])
        attn_stream([mk_std(7)], step_hooks=mlp_stats_hook)
        for m in range(4):
            ps = (pav if m == 2 else pd).tile(
                [128, 1024], F32, tag=("pd", "pd", "pav", "pd")[m],
                name=f"mlp_ps{m}")
            for qt in range(2):
                s = slice(qt * 512, (qt + 1) * 512)
                for kk in range(4):
                    nc.tensor.matmul(ps[:, s], nlw[:, kk, m * 128:(m + 1) * 128],
                                     mlp_in[:, kk, s], start=(kk == 0),
                                     stop=(kk == 3))
            nc.scalar.activation(mlpT[:, m, :], ps[:], sim_gelu,
                                 bias=bcol("nlb")[:, m:m + 1], scale=1.0)
            nc.vector.scalar_tensor_tensor(
                out=comb[:, m, :], in0=mlpT[:, m, :],
                scalar=bcol("negsub")[:, m:m + 1], in1=outT_std[:, m, :],
                op0=ALU.mult, op1=ALU.add)

        # ---------------- output projection ----------------
        outT_r = t["outT"].rearrange("(c p) n -> p c n", p=128)
        for m in range(4):
            ps = (pav if m == 2 else pd).tile(
                [128, 1024], F32, tag=("pd", "pd", "pav", "pd")[m],
                name=f"wo_ps{m}")
            for qt in range(2):
                s = slice(qt * 512, (qt + 1) * 512)
                for kk in range(4):
                    nc.tensor.matmul(ps[:, s], wout[:, kk, m * 128:(m + 1) * 128],
                                     comb[:, kk, s], start=(kk == 0), stop=(kk == 3))
                nc.vector.tensor_scalar_add(osb[:, m, s], ps[:, s],
                                            bcol("bout")[:, m:m + 1])
                (nc.sync if (2 * m + qt) % 2 == 0 else nc.scalar).dma_start(
                    outT_r[:, m, s], osb[:, m, s])

    nc.compile()
    return nc


def make_in_maps(x, ck, cv, w_qkv, w_out, b_out,
                 ckw0, ckb0, ckw1, ckb1, ckw2, ckb2,
                 cvw0, cvb0, cvw1, cvb1, cvw2, cvb2,
                 nl_w, nl_b, sub_ratio):
    bf = lambda a: np.ascontiguousarray(np.asarray(a, np.float32)).astype(NPBF16)
    rows = np.asarray(cvb2, np.float32).reshape(1, 512).astype(NPBF16)
    smallcols = np.zeros((128, 32), np.float32)
    for nm, arr in (("nlb", nl_b), ("bout", b_out),
                    ("ckb0", ckb0), ("ckb1", ckb1), ("ckb2", ckb2),
                    ("cvb0", cvb0), ("cvb1", cvb1)):
        smallcols[:, COLS[nm]:COLS[nm] + 4] = \
            np.asarray(arr, np.float32).reshape(4, 128).T
    smallcols[:, 8:12] = -np.asarray(sub_ratio, np.float32).reshape(4, 128).T
    shared = {
        "wqkvT": bf(w_qkv.T),
        "ckw0T": bf(ckw0.T), "ckw1T": bf(ckw1.T), "ckw2T": bf(ckw2.T),
        "cvw0T": bf(cvw0.T), "cvw1T": bf(cvw1.T), "cvw2T": bf(cvw2.T),
        "nlwT": bf(nl_w.T), "woutT": bf(w_out.T),
        "rows": rows, "smallcols": smallcols,
        "ones_blk": np.ones((128, 512), NPBF16),
    }
    in_maps = []
    for b in range(B):
        m = dict(shared)
        m["xT"] = bf(x[b].T)
        m["ckT"] = bf(ck[b].transpose(0, 2, 1).reshape(CH * CD, N))
        m["cvT"] = bf(cv[b].transpose(0, 2, 1).reshape(CH * CD, N))
        in_maps.append(m)
    return in_maps


_NC_CACHE = {}


def get_nc():
    if "nc" not in _NC_CACHE:
        _NC_CACHE["nc"] = build_nc()
    return _NC_CACHE["nc"]


def kernel(**inputs):
    inputs = {k: np.asarray(v) for k, v in inputs.items()}
    nc = get_nc()
    in_maps = make_in_maps(**inputs)
    res = run_bass_kernel_spmd(nc, in_maps, list(range(B)))
    out = np.empty((B, N, D), np.float32)
    for b in range(B):
        out[b] = res.results[b]["outT"].T
    return out
